# revision 62
# baseline (speedup 1.0000x reference)
"""Haar wavelet (2x2 block) decomposition kernel for 8 Trainium2 NeuronCores.

Input  x: [16, 32, 512, 512] f32
Output  : [16, 128, 256, 256] f32 = concat([pooled, diffH, diffV, diffD], axis=1)

Sharding: pure data parallel over the batch axis — core i handles batches
[2i, 2i+1] (64 images of 512x512 per core).

Per-core dataflow (all fp32), `ipi` images per iteration, P = 128/ipi
partitions per image, R = 512/P input rows per partition:
  load X [128, R*512]   (one contiguous R*512*4-byte run per partition)
  s = E + O, d = E - O          (row butterfly, DVE)
  po = (s_e + s_o) * 0.25       (column butterfly, DVE + ACT scale)
  dv = (s_e - s_o) * 0.5
  dh = (d_e + d_o) * 0.5
  dd =  d_e - d_o
  one fused store of all 4 planes (4 contiguous runs per partition)
With inplace=True the output overwrites the X tile (X is dead after the
row butterfly), halving SBUF footprint so more buffers fit.

Tuning history (slope-protocol HW measurements, see bench.py/compare.py):
the kernel is HBM-bound.  Per-NC rates measured via DMA-only variants:
pure loads 345 GB/s, pure stores ~340 GB/s, but mixed 50/50 R+W traffic
only ~323 GB/s — and that mixed-traffic rate is the wall: a DMA-only
kernel (no compute) times identically to the full kernel, store
descriptor structure is irrelevant (a perfectly-linear store AP times
the same as the 4-runs-per-partition real one), and forcing coarse
unidirectional bursts by putting both directions on one HWDGE ring in
FIFO alternation does not help.  What did help: ipi 2 -> 4 (2 MiB -> 4
MiB DMAs) and deeper X buffering (bufs 3 -> 5, enabled by in-place
output reuse), worth ~7% combined in an interleaved A/B measurement
(444.7 -> 413.9 us/core; re-confirmed 447.0 -> 417.2).  Everything else
measured worse or neutral: ipi=8 (even with 3 bufs via chunk-major O +
per-chunk stores: 418.6 vs 412.0), deeper ipi=2 buffering (427.8),
bufs=6 (reachable via the SWDGE-scratch shave + bf16 s/d but only 0.6 us
faster — depth saturates at 5), grouped unidirectional R/W phases
(423.6), ramp shaping (413.0 vs 408.0), tail-chunking the final
iteration (420.8 vs 407.4), ring swap/alternation, and every
finer-than-4-MiB store granularity.  SBUF usable is 207.87
KiB/partition; DVE cannot write bf16 to PSUM (NCC_IBIR311); DMA APs
balance at most 3 free dims, which is why inplace + chunks>1 needs one
store per chunk.

The walrus build available here only accepts ONE sync-wait per instruction
(setupSyncWait: "Too many sync wait commands"), while Tile freely attaches
several.  _split_multi_waits() post-processes the serialized BIR, hoisting
all-but-one wait of every instruction onto single-wait NoOps inserted just
before it (same engine, so per-engine program order is preserved).
"""

import functools

import numpy as np
import orjson

import concourse.bass as bass
import concourse.mybir as mybir
from concourse.tile import TileContext

_N_CORES = 8
_B, _C, _H, _W = 16, 32, 512, 512
_BPC = _B // _N_CORES  # batches per core
_IMGS = _BPC * _C  # images per core
_F32 = mybir.dt.float32

# default per-core pipeline config (see _build_nc)
_DEF = dict(ipi=4, inplace=True, bufs=5, sd_bufs=1, o_bufs=2, sd_bf16=False)


def _split_multi_waits(j: dict) -> dict:
    for fn in j["functions"]:
        for blk in fn["blocks"]:
            out = []
            for ins in blk["instructions"]:
                si = ins.get("sync_info")
                waits = (si or {}).get("on_wait") or []
                if len(waits) > 1:
                    for k, w in enumerate(waits[:-1]):
                        out.append(
                            {
                                "debug": ins.get("debug", 0),
                                "engine": ins["engine"],
                                "ins": [],
                                "outs": [],
                                "name": f"{ins['name']}__w{k}",
                                "opcode": "NoOp",
                                "text_hint": "split_wait",
                                "sync_info": {"on_update": [], "on_wait": [w]},
                            }
                        )
                    si["on_wait"] = [waits[-1]]
                out.append(ins)
            blk["instructions"] = out
    return j


if not getattr(bass.Bass.to_json_bytes, "_haar_split_patch", False):
    _orig_to_json_bytes = bass.Bass.to_json_bytes

    def _patched_to_json_bytes(self):
        j = orjson.loads(_orig_to_json_bytes(self))
        _split_multi_waits(j)
        return orjson.dumps(j)

    _patched_to_json_bytes._haar_split_patch = True
    bass.Bass.to_json_bytes = _patched_to_json_bytes


@functools.lru_cache(maxsize=None)
def _build_nc(
    reps=1, ipi=None, inplace=None, bufs=None, sd_bufs=None, o_bufs=None, sd_bf16=None,
    mode="full", chunks=None, rings=0, sd_psum=False, group=None, ramp=None,
    shave=0, tail=0,
) -> bass.Bass:
    ipi = _DEF["ipi"] if ipi is None else ipi
    inplace = _DEF["inplace"] if inplace is None else inplace
    bufs = _DEF["bufs"] if bufs is None else bufs
    sd_bufs = _DEF["sd_bufs"] if sd_bufs is None else sd_bufs
    o_bufs = _DEF["o_bufs"] if o_bufs is None else o_bufs
    sd_bf16 = _DEF["sd_bf16"] if sd_bf16 is None else sd_bf16
    chunks = _DEF.get("chunks", 1) if chunks is None else chunks
    group = _DEF.get("group", 0) if group is None else group
    ramp = _DEF.get("ramp", 0) if ramp is None else ramp
    sd_dt = mybir.dt.bfloat16 if sd_bf16 else _F32

    P = 128 // ipi  # partitions per image
    R = _H // P  # input rows per partition
    A = R // 2  # output rows (row-pairs) per partition
    FW = A * _W  # free size of s/d per partition
    HP = FW // 2  # free size of one output plane per partition

    # Note: bufs=6 IS reachable by shaving the (unused, HWDGE-only kernel)
    # SWDGE scratch via Bass(dynamic_dma_scratch_size=16384-512) + bf16 s/d,
    # and it runs correctly (rel 2.5e-3) — but measured only 0.6 us faster
    # interleaved (412.8 vs 413.4): the buffer-depth benefit saturates at 5,
    # so not worth the accuracy cost.
    # shave: reclaim bytes from the SWDGE descriptor-ring scratch (never
    # written — this kernel is HWDGE-only) for configs right at the SBUF cap
    nc = bass.Bass(dynamic_dma_scratch_size=16384 - (512 if shave else 0))
    x = nc.dram_tensor("x", [_IMGS, _H, _W], _F32, kind="ExternalInput")
    y = nc.dram_tensor("y", [4 * _IMGS, _H // 2, _W // 2], _F32, kind="ExternalOutput")
    yv = y.rearrange("(b k c) h w -> b c k (h w)", b=_BPC, k=4)

    import contextlib

    with TileContext(nc) as tc:
        with (
            tc.tile_pool(name="sbuf", bufs=bufs) as pool,
            (
                tc.psum_pool(name="psum", bufs=1)
                if sd_psum
                else contextlib.nullcontext(None)
            ) as ppool,
        ):
            sd_pool = ppool if sd_psum else pool

            def load(img0, ld_eng, n=None):
                n = ipi if n is None else n
                Pn, Rn = 128 // n, _H // (128 // n)
                # same tag as the full-size tiles: shares the slot rotation
                X = pool.tile([128, Rn * _W], _F32, tag="X", name="X")
                ld_eng.dma_start(
                    out=X,
                    in_=x[img0 : img0 + n].rearrange(
                        "i (p a) w -> (i p) (a w)", p=Pn, a=Rn
                    ),
                )
                return X

            def compute_and_store(img0, X, st_eng, n=None, chunks_n=None):
                chunks_n = chunks if chunks_n is None else chunks_n
                n = ipi if n is None else n
                Pn = 128 // n
                Rn = _H // Pn
                An = Rn // 2
                FWn = An * _W
                HPn = FWn // 2
                O = (
                    X
                    if inplace
                    else pool.tile([128, Rn * _W], _F32, tag="O", bufs=o_bufs, name="O")
                )
                # inplace + chunks>1: chunk-major O (each chunk overwrites
                # only the X region it just consumed) with one store PER
                # CHUNK — the per-chunk store AP is 3-dim so it balances,
                # unlike the fused chunk-major store.
                per_chunk_store = inplace and chunks_n > 1
                ca = An // chunks_n
                cs = ca * _W // 2
                b, c0 = divmod(img0, _C)
                yvi = yv[b, c0 : c0 + n].rearrange("i k (p aw) -> (i p) k aw", p=Pn)
                for t in range(chunks_n):
                    Xc = X[:, t * ca * 2 * _W : (t + 1) * ca * 2 * _W]
                    Xv = Xc.rearrange("q (a eo w) -> q eo a w", a=ca, eo=2)
                    s = sd_pool.tile(
                        [128, ca * _W], sd_dt, tag="s", bufs=sd_bufs, name="s"
                    )
                    d = sd_pool.tile(
                        [128, ca * _W], sd_dt, tag="d", bufs=sd_bufs, name="d"
                    )
                    nc.vector.tensor_add(out=s, in0=Xv[:, 0], in1=Xv[:, 1])
                    nc.vector.tensor_sub(out=d, in0=Xv[:, 0], in1=Xv[:, 1])
                    sr = s.rearrange("q (x v) -> q v x", v=2)
                    dr = d.rearrange("q (x v) -> q v x", v=2)
                    if inplace:  # chunk-major (chunks==1: same as plane-major)
                        sec = [(t * 4 + k) * cs for k in range(4)]
                    else:  # plane-major
                        sec = [k * HPn + t * cs for k in range(4)]
                    po = O[:, sec[0] : sec[0] + cs]
                    dh = O[:, sec[1] : sec[1] + cs]
                    dv = O[:, sec[2] : sec[2] + cs]
                    dd = O[:, sec[3] : sec[3] + cs]
                    nc.vector.tensor_add(out=po, in0=sr[:, 0], in1=sr[:, 1])
                    nc.vector.tensor_add(out=dh, in0=dr[:, 0], in1=dr[:, 1])
                    nc.vector.tensor_sub(out=dv, in0=sr[:, 0], in1=sr[:, 1])
                    nc.vector.tensor_sub(out=dd, in0=dr[:, 0], in1=dr[:, 1])
                    nc.scalar.mul(po, po, 0.25)
                    nc.scalar.mul(dh, dh, 0.5)
                    nc.scalar.mul(dv, dv, 0.5)
                    if per_chunk_store:
                        st_eng.dma_start(
                            out=yvi[:, :, t * cs : (t + 1) * cs],
                            in_=O[:, t * 4 * cs : (t + 1) * 4 * cs].rearrange(
                                "q (k c) -> q k c", k=4
                            ),
                        )
                if not per_chunk_store:
                    st_eng.dma_start(
                        out=yvi,
                        in_=O.rearrange("q (k aw) -> q k aw", k=4),
                    )

            def grouped_body():
                # Phase-separated R/W: all of a group's loads, then all of
                # its stores, on the SAME HWDGE ring — the FIFO prevents
                # group k+1's loads from draining before group k's stores,
                # so HBM sees ~group*4 MiB unidirectional bursts (pure-read
                # 345 GB/s and pure-write ~350 GB/s vs 323 GB/s for the
                # packet-interleaved 50/50 mix).
                assert group <= bufs
                idxs = list(range(0, _IMGS, ipi))
                for g0 in range(0, len(idxs), group):
                    xs = [(i0, load(i0, nc.sync)) for i0 in idxs[g0 : g0 + group]]
                    for i0, X in xs:
                        compute_and_store(i0, X, nc.sync)

            def body():
                if mode == "noop":
                    # one tiny op: slope of this measures the For_i
                    # per-iteration overhead (all-engine barrier + sem reset)
                    z = pool.tile([128, 16], _F32, tag="z", bufs=1)
                    nc.vector.memset(z, 0.0)
                    return
                if mode == "full" and group:
                    grouped_body()
                    return
                if mode == "full" and ramp:
                    # finer first/last iterations: the first store becomes
                    # eligible sooner (shorter read-only ramp) and the final
                    # store tail halves (shorter write-only drain)
                    sched = [2, 2] + [ipi] * ((_IMGS - 8) // ipi) + [2, 2]
                    img0 = 0
                    for n in sched:
                        X = load(img0, nc.sync, n)
                        compute_and_store(img0, X, nc.scalar, n)
                        img0 += n
                    return
                # Loads go on the SP HWDGE ring, stores on the ACT ring so
                # both rings drive the SDMA pool concurrently.
                x_tiles = []
                for img0 in range(0, _IMGS, ipi):
                    if mode in ("stores", "storespure") and img0 >= bufs * ipi:
                        X = x_tiles[(img0 // ipi) % bufs]
                    else:
                        X = pool.tile([128, R * _W], _F32, tag="X")
                        x_tiles.append(X)
                        if mode == "storespure":
                            nc.vector.memset(X, 0.0)
                    it = img0 // ipi
                    # rings: 0 = loads on SP, stores on ACT; 1 = swapped;
                    # 2 = alternate both by iteration parity
                    if rings == 0:
                        ld_eng, st_eng = nc.sync, nc.scalar
                    elif rings == 1:
                        ld_eng, st_eng = nc.scalar, nc.sync
                    else:
                        ld_eng = nc.sync if it % 2 == 0 else nc.scalar
                        st_eng = nc.scalar if it % 2 == 0 else nc.sync
                    if mode in ("full", "loads", "dma", "dmaser", "dmalin") or (
                        mode == "stores" and img0 < bufs * ipi
                    ):
                        ld_eng.dma_start(
                            out=X,
                            in_=x[img0 : img0 + ipi].rearrange(
                                "i (p a) w -> (i p) (a w)", p=P, a=R
                            ),
                        )
                    if mode in ("stores", "storespure", "dma", "dmaser"):
                        b, c0 = divmod(img0, _C)
                        eng = nc.sync if mode == "dmaser" else nc.scalar
                        eng.dma_start(
                            out=yv[b, c0 : c0 + ipi].rearrange(
                                "i k (p aw) -> (i p) k aw", p=P
                            ),
                            in_=X.rearrange("q (k aw) -> q k aw", k=4),
                        )
                    if mode == "dmalin":
                        # timing probe: same bytes, one contiguous run/partition
                        g = img0 // ipi
                        ylin = y.rearrange("(g a) h w -> g (a h) w", a=4 * ipi)[
                            g
                        ].rearrange("(p r) w -> p (r w)", p=128)
                        nc.scalar.dma_start(out=ylin, in_=X[:, : ylin.shape[1]])
                    if mode != "full":
                        continue
                    # tail: chunk the final iteration so its store overlaps
                    # its compute, shortening the one-shot drain tail
                    last = img0 + ipi >= _IMGS
                    compute_and_store(
                        img0, X, st_eng, chunks_n=tail if (tail and last) else None
                    )

            if reps == 1:
                body()
            else:
                # HW repeat loop for slope-based timing (hw_slope.py)
                with tc.For_i(0, reps):
                    body()
    return nc


@functools.lru_cache(maxsize=None)
def _build_runner(
    reps=1, ipi=None, inplace=None, bufs=None, sd_bufs=None, o_bufs=None, sd_bf16=None
):
    """Compile once; return a callable shards -> full output.

    Mirrors bass2jax.run_bass_via_pjrt's multi-core path (shard_map over the
    8 axon devices, donated zero output buffers), but keeps the jitted
    function alive so repeated kernel() calls don't recompile the NEFF.
    """
    import jax
    from jax.sharding import Mesh, PartitionSpec, NamedSharding
    from jax.experimental.shard_map import shard_map
    from concourse import bass2jax

    nc = _build_nc(reps, ipi, inplace, bufs, sd_bufs, o_bufs, sd_bf16)
    partition_name = nc.partition_id_tensor.name if nc.partition_id_tensor else None
    in_names, out_names, out_avals = [], [], []
    for alloc in nc.m.functions[0].allocations:
        if not isinstance(alloc, mybir.MemoryLocationSet):
            continue
        name = alloc.memorylocations[0].name
        if alloc.kind == "ExternalInput":
            if name != partition_name:
                in_names.append(name)
        elif alloc.kind == "ExternalOutput":
            out_names.append(name)
            out_avals.append(
                jax.core.ShapedArray(
                    tuple(alloc.tensor_shape), mybir.dt.np(alloc.dtype)
                )
            )
    n_params = len(in_names)
    n_outs = len(out_names)
    all_in_names = in_names + out_names + ([partition_name] if partition_name else [])

    def _body(*args):
        operands = list(args)
        if partition_name is not None:
            operands.append(bass2jax.partition_id_tensor())
        outs = bass2jax._bass_exec_p.bind(
            *operands,
            out_avals=tuple(out_avals),
            in_names=tuple(all_in_names),
            out_names=tuple(out_names),
            lowering_input_output_aliases=(),
            sim_require_finite=True,
            sim_require_nnan=True,
            nc=nc,
        )
        return tuple(outs)

    bass2jax.install_neuronx_cc_hook()
    devices = jax.devices()[:_N_CORES]
    assert len(devices) == _N_CORES, f"need {_N_CORES} devices, got {len(devices)}"
    mesh = Mesh(np.asarray(devices), ("core",))
    in_specs = (PartitionSpec("core"),) * (n_params + n_outs)
    out_specs = (PartitionSpec("core"),) * n_outs
    sharded = jax.jit(
        shard_map(
            _body, mesh=mesh, in_specs=in_specs, out_specs=out_specs, check_rep=False
        ),
        donate_argnums=tuple(range(n_params, n_params + n_outs)),
        keep_unused=True,
    )
    out_shape = out_avals[0].shape
    zero_shape = (_N_CORES * out_shape[0], *out_shape[1:])
    sh = NamedSharding(mesh, PartitionSpec("core"))
    # allocate + fill the donated output buffer on-device: avoids a 512 MiB
    # host->device transfer of zeros per call
    make_zeros = jax.jit(
        lambda: jax.numpy.zeros(zero_shape, np.float32), out_shardings=sh
    )

    # The kernel writes every output element, so the donated buffer's
    # contents never matter — re-donate the previous call's (already
    # host-copied) output to skip the 512 MiB device zero-fill on repeat
    # calls; only the first call pays for make_zeros().
    state = {"buf": None}

    def run(x_global: np.ndarray) -> np.ndarray:
        if state["buf"] is None:
            state["buf"] = make_zeros()
        (out,) = sharded(x_global, state["buf"])
        result = np.asarray(out)
        state["buf"] = out
        return result

    return run


def kernel(x) -> np.ndarray:
    x = np.ascontiguousarray(np.asarray(x), dtype=np.float32)
    assert x.shape == (_B, _C, _H, _W), x.shape
    x_global = x.reshape(_N_CORES * _IMGS, _H, _W)  # view, no copy
    out = _build_runner()(x_global)  # [8*4*_IMGS, 256, 256], core-major
    return out.reshape(_B, 4 * _C, _H // 2, _W // 2)
